# revision 67
# baseline (speedup 1.0000x reference)
"""Trainium2 Bass kernel for nn_ClassicalQuantumAttention (optimized).

Data-parallel over batch: 128 batch elems -> 16 per NeuronCore x 8 cores.

Quantum stage: batched state tiles ST [128 nc, 128*NB], f = q*NB + b,
q = ri*64 + a.  Each rotation gate = 4 tensor_tensor ops (2 half sigma-mults
reading permuted state views with +/-sin coefficient broadcasts, 1 full
cos-mult, 1 full add); CRX = 4 ops on the control=1 half.  Coefficients
come from co [128, 60*NB] (cos) and spm [128, 120*NB] (+sin | -sin) tiles
via stride-0 broadcast views; no per-gate expansion copies.

Pool (gpsimd) group runs batch elems 0-1 and starts right after their
classical params are ready; DVE (vector) group runs elems 2-15 in fp16.

qff ansatz + expvals folded on host into 19 symmetric 128x128 matrices;
LCU mixing is one K=128 matmul per 512-wide chunk.
"""

import numpy as np
import sys

for _p in ("/opt/trn_rl_repo",):
    if _p not in sys.path:
        sys.path.insert(0, _p)

import concourse.bass as bass
import concourse.tile as tile
from concourse import mybir
from concourse.bass_utils import run_bass_kernel_spmd

F32 = mybir.dt.float32
F16 = mybir.dt.float16
ALU = mybir.AluOpType
AF = mybir.ActivationFunctionType
AX = mybir.AxisListType

N_CORES = 8
B_TOT = 128
BPC = B_TOT // N_CORES  # 16
C_IN = 64
T = 2048
D = 256
CH = 16
NC = T // CH  # 128
NQ = 6
DIM = 64

# (engine_attr, b_start, NB, state_dtype) — Pool group first so its quantum
# stage starts as soon as its classical params are ready.  DVE NB must be
# EVEN: fp16 2x mode needs 4B-aligned unit-stride runs (runs are NB elems).
GROUPS = [("gpsimd", 0, 2, F32), ("vector", 2, 14, F16)]


def ansatz_gates(n_layers):
    gates = []
    idx = 0
    for _ in range(n_layers):
        for i in range(NQ):
            gates.append(("rx", i, idx))
            gates.append(("ry", i, idx + 1))
            gates.append(("rz", i, idx + 2))
            idx += 3
        for i in range(NQ):
            gates.append(("crx", (i, (i + 1) % NQ), idx))
            idx += 1
        for i in range(NQ - 1, -1, -1):
            gates.append(("crx", (i, (i - 1) % NQ), idx))
            idx += 1
    return gates


# --------------------------------------------------------------- AP helpers
def rawv(t, elem_off, dims):
    return bass.AP(tensor=t.tensor, offset=t.offset + elem_off,
                   ap=[list(t.ap[0])] + dims)


def mk(axes):
    """Merge (stride, count) axes outer->inner: drop count-1 axes, merge
    adjacent when outer.stride == inner.stride*inner.count."""
    dims = []
    for s, c in axes:
        if c == 1:
            continue
        if dims and dims[-1][0] == s * c:
            dims[-1] = [s, dims[-1][1] * c]
        else:
            dims.append([s, c])
    if not dims:
        dims = [[1, 1]]
    assert len(dims) <= 3, f"mk dims {dims}"
    return dims


def _split_multi_waits(nc):
    """Walrus build allows at most ONE sync-wait per instruction; hoist
    extra waits onto same-engine NoOps."""
    ctr = [0]
    for f in nc.m.functions:
        for b in f.blocks:
            new = []
            for inst in b.instructions:
                si = inst.sync_info
                if si is not None and len(si.on_wait) > 1:
                    waits = list(si.on_wait)
                    for w in waits[:-1]:
                        ctr[0] += 1
                        nop = mybir.InstNoOp(
                            name=f"wsplit-{ctr[0]}",
                            ins=[], outs=[],
                            engine=inst.engine,
                            sync_info=mybir.SyncInfo(on_wait=[w], on_update=[]),
                        )
                        new.append(nop)
                    inst.sync_info = mybir.SyncInfo(
                        on_wait=[waits[-1]], on_update=list(si.on_update)
                    )
                new.append(inst)
            b.instructions = new


# ----------------------------------------------------------- gate emission
def emit_gates(g):
    """Emit the full 2-layer ansatz for one group."""
    eng, NB = g["eng"], g["NB"]
    ST, TMP, co, spm = g["ST"], g["TMP"], g["co"], g["spm"]
    sNB = 60 * NB  # -sin block offset inside spm
    eng.memset(ST, 0.0)
    eng.memset(ST[:, 0:NB], 1.0)

    def sview(t, p, L, ri=None, k=None, ri_neg=False, k_neg=False):
        off = 0
        axes = []
        if ri is None:
            if ri_neg:
                axes.append((-64 * NB, 2))
                off += 64 * NB
            else:
                axes.append((64 * NB, 2))
        else:
            off += ri * 64 * NB
        axes.append(((1 << (p + 1)) * NB, 1 << (5 - p)))
        if k is None:
            s = (1 << p) * NB
            if k_neg:
                axes.append((-s, 2))
                off += s
            else:
                axes.append((s, 2))
        else:
            off += k * (1 << p) * NB
        axes.append((1, L))
        return rawv(t, off, mk(axes))

    def cf(tile_, base, rest, slot=None):
        dims = []
        if slot is not None:
            dims.append(list(slot))
        if rest > 1:
            dims.append([0, rest])
        dims.append([1, NB])
        return rawv(tile_, base, dims)

    def crx_view(t, pc, pt, ri, kt_neg):
        off = (1 << pc) * NB
        axes = []
        if ri is None:
            axes.append((64 * NB, 2))
        else:
            off += ri * 64 * NB
        for bit in range(5, -1, -1):
            if bit == pc:
                continue
            s = (1 << bit) * NB
            if bit == pt and kt_neg:
                axes.append((-s, 2))
                off += s
            else:
                axes.append((s, 2))
        axes.append((1, NB))
        return rawv(t, off, mk(axes))

    for gi, (kind, loc, j) in enumerate(ansatz_gates(2)):
        jb = j * NB
        if kind != "crx":
            p = 5 - loc
            ah = (gi // 3) + 1 if gi < 18 else 6
            la = p - (6 - ah)
            assert la >= 0
            L = NB << la
            H = 1 << (5 - p)
            LN_ = L // NB  # lo amp count
            expand = (kind == "rx" and gi < 18)
            if kind == "rx":
                for ri_o in (0, 1):
                    ov = sview(TMP, p, L, ri=ri_o, k=(1 if expand else None))
                    iv = sview(ST, p, L, ri=1 - ri_o,
                               k=(0 if expand else None),
                               k_neg=not expand)
                    c = cf(spm, jb + (0 if ri_o == 0 else sNB),
                           H * (1 if expand else 2) * LN_)
                    eng.tensor_tensor(ov, iv, c, ALU.mult)
            elif kind == "ry":
                for k_o in (0, 1):
                    ov = sview(TMP, p, L, k=k_o)
                    iv = sview(ST, p, L, k=1 - k_o)
                    c = cf(spm, jb + (sNB if k_o == 0 else 0), 2 * H * LN_)
                    eng.tensor_tensor(ov, iv, c, ALU.mult)
            else:  # rz
                for k_o in (0, 1):
                    ov = sview(TMP, p, L, k=k_o)
                    iv = sview(ST, p, L, k=k_o, ri_neg=True)
                    base = jb + (0 if k_o == 0 else sNB)
                    slot = [sNB if k_o == 0 else -sNB, 2]
                    c = cf(spm, base, H * LN_, slot=slot)
                    eng.tensor_tensor(ov, iv, c, ALU.mult)
            if expand:
                cm = sview(ST, p, L, k=0)
                eng.tensor_tensor(cm, cm, cf(co, jb, 2 * H * LN_), ALU.mult)
                ao = sview(ST, p, L, k=1)
                ai = sview(TMP, p, L, k=1)
                eng.tensor_tensor(ao, ao, ai, ALU.add)
            else:
                cm = sview(ST, p, L)
                eng.tensor_tensor(cm, cm, cf(co, jb, 4 * H * LN_), ALU.mult)
                ao = sview(ST, p, L)
                ai = sview(TMP, p, L)
                eng.tensor_tensor(ao, ao, ai, ALU.add)
        else:
            wc, wt = loc
            pc, pt = 5 - wc, 5 - wt
            for ri_o in (0, 1):
                ov = crx_view(TMP, pc, pt, ri_o, False)
                iv = crx_view(ST, pc, pt, 1 - ri_o, True)
                c = cf(spm, jb + (0 if ri_o == 0 else sNB), 32)
                eng.tensor_tensor(ov, iv, c, ALU.mult)
            cm = crx_view(ST, pc, pt, None, False)
            eng.tensor_tensor(cm, cm, cf(co, jb, 64), ALU.mult)
            ao = crx_view(ST, pc, pt, None, False)
            ai = crx_view(TMP, pc, pt, None, False)
            eng.tensor_tensor(ao, ao, ai, ALU.add)


# ----------------------------------------------------------------- program
def build_program(split_waits=True):
    nc = bass.Bass()

    for vconst in (float(np.pi / 2), 1e-5, 0.25, float(0.25 + np.pi / 2)):
        t = nc.alloc_sbuf_tensor(f"const-f32-{vconst}", [128, 1], F32)
        nc.gpsimd.memset(t.ap(), vconst)
        nc.const_aps.aps[(F32, vconst)] = t.ap()
    nc.all_engine_barrier()

    xs = nc.declare_dram_parameter("xs", [BPC, C_IN, T], F16, isOutput=False)
    xq = nc.declare_dram_parameter("xq", [BPC, NC, CH * C_IN], F16, isOutput=False)
    wfb = nc.declare_dram_parameter("wfb", [C_IN + 1, 128], F16, isOutput=False)
    aw2 = nc.declare_dram_parameter("aw2", [128, 1], F16, isOutput=False)
    epw = nc.declare_dram_parameter("epw", [C_IN + 1, 60], F16, isOutput=False)
    cf2 = nc.declare_dram_parameter("cf2", [NC, 2], F32, isOutput=False)
    mt = nc.declare_dram_parameter("mt", [128, 19 * 128], F16, isOutput=False)
    owb = nc.declare_dram_parameter("owb", [19, D], F32, isOutput=False)
    cw1 = nc.declare_dram_parameter("cw1", [128, 2 * D], F32, isOutput=False)
    cb1 = nc.declare_dram_parameter("cb1", [1, D], F32, isOutput=False)
    cw2 = nc.declare_dram_parameter("cw2", [128, 4], F32, isOutput=False)
    cb2 = nc.declare_dram_parameter("cb2", [1, 2], F32, isOutput=False)
    idn = nc.declare_dram_parameter("idn", [128, 128], F32, isOutput=False)
    out = nc.declare_dram_parameter("out", [BPC, 2], F32, isOutput=True)

    with tile.TileContext(nc) as tc:
        with (
            tc.tile_pool(name="const", bufs=1) as cp,
            tc.tile_pool(name="xbuf", bufs=4) as xpool,
            tc.tile_pool(name="xqbuf", bufs=4) as xqpool,
            tc.tile_pool(name="tanh", bufs=6) as thpool,
            tc.tile_pool(name="sscb", bufs=2) as sscpool,
            tc.tile_pool(name="small", bufs=8) as sm,
            tc.tile_pool(name="ps_h", bufs=2, space="PSUM") as ps_h,
            tc.tile_pool(name="ps_s", bufs=2, space="PSUM") as ps_s,
            tc.tile_pool(name="ps_m", bufs=2, space="PSUM") as ps_m,
            tc.tile_pool(name="ps_t", bufs=1, space="PSUM") as ps_t,
            tc.tile_pool(name="ps_x", bufs=1, space="PSUM") as ps_x,
        ):
            def cload(name, dram, shape, dt=F32):
                t = cp.tile(shape, dt, tag=name, name=name)
                nc.sync.dma_start(out=t, in_=dram[:, :])
                return t

            wfb_s = cload("wfb", wfb, [C_IN + 1, 128], F16)
            aw2_s = cload("aw2", aw2, [128, 1], F16)
            epw_s = cload("epw", epw, [C_IN + 1, 60], F16)
            idn_s = cload("idn", idn, [128, 128])
            idn_h = cp.tile([128, 128], F16, tag="idn_h")
            nc.vector.tensor_copy(idn_h, idn_s)
            # tail-only weights: keep their DMAs off the startup window
            with tc.tile_wait_until(0.5):
                cf2_s = cload("cf2", cf2, [NC, 2])
                mt_s = cload("mt", mt, [128, 19 * 128], F16)
                owb_s = cload("owb", owb, [19, D])
                cw1_s = cload("cw1", cw1, [128, 2 * D])
                cb1_s = cload("cb1", cb1, [1, D])
                cw2_s = cload("cw2", cw2, [128, 4])
                cb2_s = cload("cb2", cb2, [1, 2])

            ones = cp.tile([1, 128], F32, tag="ones")
            nc.vector.memset(ones, 1.0)
            ones_col = cp.tile([128, 1], F32, tag="ones_col")
            nc.vector.memset(ones_col, 1.0)

            sc_all = cp.tile([NC, BPC * CH], F16, tag="sc_all", name="sc_all")
            esc = cp.tile([NC, BPC * CH], F32, tag="esc", name="esc")
            w_all = cp.tile([NC, BPC * CH], F16, tag="w_all", name="w_all")

            # per-engine-group quantum tiles
            grp = []
            for gi_, (engname, b0, NB, sdt) in enumerate(GROUPS):
                g = dict(
                    eng=getattr(nc, engname), b0=b0, NB=NB, dt=sdt,
                    ST=cp.tile([128, 128 * NB], sdt, tag=f"ST{gi_}", name=f"ST{gi_}"),
                    TMP=cp.tile([128, 128 * NB], sdt, tag=f"TMP{gi_}", name=f"TMP{gi_}"),
                    co=cp.tile([128, 60 * NB], sdt, tag=f"co{gi_}", name=f"co{gi_}"),
                    spm=cp.tile([128, 120 * NB], sdt, tag=f"spm{gi_}", name=f"spm{gi_}"),
                )
                grp.append(g)
            cf2_h = cp.tile([NC, 2], F16, tag="cf2h")
            with tc.tile_wait_until(0.5):
                nc.vector.tensor_copy(cf2_h, cf2_s)

            x_sb = [xpool.tile([C_IN + 1, T], F16, tag="x", name=f"xsb{i}") for i in range(4)]
            xq_sb = [xqpool.tile([NC, CH * C_IN], F16, tag="xq", name=f"xqsb{i}") for i in range(4)]
            nc.vector.memset(x_sb[0][C_IN : C_IN + 1, :], 1.0)
            for i in (1, 2, 3):
                nc.gpsimd.memset(x_sb[i][C_IN : C_IN + 1, :], 1.0)

            xwt_sb = [sm.tile([C_IN + 1, NC], F16, tag=f"xwt{i}", name=f"xwt{i}") for i in range(3)]
            for i in range(3):
                nc.vector.memset(xwt_sb[i][C_IN : C_IN + 1, :], 1.0)

            lqs_g = [cp.tile([2, 128 * nbv], F32, tag=f"lqs{i_}", name=f"lqs{i_}")
                     for i_, (_, _, nbv, _) in enumerate(GROUPS)]
            vTr = cp.tile([128, BPC], F32, tag="vTr")
            vTi = cp.tile([128, BPC], F32, tag="vTi")
            vT = cp.tile([128, BPC], F32, tag="vT")
            vT_h = cp.tile([128, BPC], F16, tag="vTh")
            prod = cp.tile([128, 19 * BPC], F32, tag="prod")
            qrow = cp.tile([1, 19 * BPC], F32, tag="qrow")
            qfT = cp.tile([19, BPC], F32, tag="qfT")
            ssc_n = cp.tile([BPC, 1], F32, tag="ssc")
            rss = cp.tile([BPC, 1], F32, tag="rss")

            par_sb = [None] * BPC

            # ===================== classical passes ======================
            def pass1(b):
                xb = x_sb[b % 4]
                nc.sync.dma_start(out=xb[0:C_IN, :], in_=xs[b, :, :])
                # score rows land on PSUM partitions 0/32 of two tiles; each
                # pair is moved by ONE cast (partitions run in parallel on DVE)
                sc_ps = [None, None]
                ssc_t = [None, None]
                for half in range(2):
                    sc_ps[half] = ps_s.tile([128, 512], F32, tag="sc",
                                            name=f"scps{half}")
                    ssc_t[half] = sscpool.tile([33, 512], F16, tag=f"ss{half}",
                                               name=f"ss{half}")
                for blk in range(4):
                    hp = ps_h.tile([128, 512], F32, tag="hp")
                    nc.tensor.matmul(hp, wfb_s, xb[:, blk * 512 : (blk + 1) * 512],
                                     start=True, stop=True)
                    th = thpool.tile([128, 512], F16, tag="th")
                    nc.scalar.activation(th, hp, AF.Tanh)
                    nc.tensor.matmul(sc_ps[blk // 2][32 * (blk % 2) : 32 * (blk % 2) + 1, :],
                                     aw2_s, th, start=True, stop=True)
                for half in range(2):
                    nc.vector.tensor_copy(ssc_t[half], sc_ps[half][0:33, :])
                for half in range(2):
                    # one DMA covers both score rows (partition-strided src)
                    st = ssc_t[half]
                    nc.sync.dma_start(
                        out=sc_all[64 * half : 64 * (half + 1),
                                   b * CH : (b + 1) * CH],
                        in_=bass.AP(tensor=st.tensor, offset=st.offset,
                                    ap=[[512 * 32, 2], [CH, 32], [1, CH]]))

            def softmax(b_lo, b_hi):
                nb = b_hi - b_lo
                lo, hi = b_lo * CH, b_hi * CH
                nc.scalar.activation(esc[:, lo:hi], sc_all[:, lo:hi], AF.Exp)
                ssum = sm.tile([NC, BPC], F32, tag="ssum")
                nc.vector.tensor_reduce(
                    ssum[:, 0:nb],
                    esc[:, lo:hi].rearrange("p (n k) -> p n k", n=nb, k=CH),
                    AX.X, ALU.add)
                rsum = sm.tile([NC, BPC], F32, tag="rsum")
                nc.vector.reciprocal(rsum[:, 0:nb], ssum[:, 0:nb])
                for bb in range(nb):
                    b = b_lo + bb
                    nc.vector.tensor_scalar_mul(
                        w_all[:, b * CH : (b + 1) * CH],
                        esc[:, b * CH : (b + 1) * CH],
                        rsum[:, bb : bb + 1])

            def pass2(b):
                xqb = xq_sb[b % 4]
                nc.sync.dma_start(out=xqb, in_=xq[b, :, :])
                # early elems ride the Pool engine (idle until its gates);
                # later ones use DVE
                xw_eng = nc.vector
                tmpxw = sm.tile([NC, CH * C_IN], F16, tag="tmpxw")
                wv = rawv(w_all, b * CH, [[0, C_IN], [1, CH]])
                xw_eng.tensor_tensor(
                    rawv(tmpxw, 0, [[CH, C_IN], [1, CH]]),
                    rawv(xqb, 0, [[CH, C_IN], [1, CH]]),
                    wv, ALU.mult)
                xw = sm.tile([NC, C_IN], F16, tag="xw")
                with nc.allow_low_precision(reason="16-wide sum, tol 2e-2"):
                    xw_eng.tensor_reduce(
                        xw, tmpxw.rearrange("p (c k) -> p c k", c=C_IN, k=CH),
                        AX.X, ALU.add)
                xwt_ps = ps_x.tile([C_IN, NC], F16, tag="mh")
                nc.tensor.transpose(xwt_ps, xw, idn_h)
                xwt = xwt_sb[b % 3]
                nc.scalar.copy(xwt[0:C_IN, :], xwt_ps)
                par = ps_t.tile([NC, 60], F32, tag="t")
                nc.tensor.matmul(par, xwt, epw_s, start=True, stop=True)
                par_s = sm.tile([NC, 60], F32, tag=f"pars{b % 8}", name=f"pars{b % 8}")
                # par_s holds tanh(p/2); sigmoid(p) = (1+tanh(p/2))/2 is
                # folded into the Sin activations (Tanh table stays resident,
                # avoiding Sigmoid table swaps mid-stream).
                nc.scalar.activation(par_s, par, AF.Tanh, scale=0.5)
                par_sb[b] = par_s

            def coeff_acts(g, bs=None):
                """sin/cos coefficient tiles for group batch elems (by func)."""
                NB, b0 = g["NB"], g["b0"]
                if bs is None:
                    bs = range(b0, b0 + NB)
                for b in bs:
                    nc.scalar.activation(
                        rawv(g["co"], b - b0, [[NB, 60]]), par_sb[b],
                        AF.Sin, bias=float(0.25 + np.pi / 2), scale=0.25)
                for b in bs:
                    nc.scalar.activation(
                        rawv(g["spm"], b - b0, [[NB, 60]]), par_sb[b],
                        AF.Sin, bias=0.25, scale=0.25)

            def coeff_neg(g):
                # -sin half of spm in one engine-side negate (off scalar)
                NB = g["NB"]
                sNB = 60 * NB
                g["eng"].tensor_scalar_mul(
                    g["spm"][:, sNB : 2 * sNB], g["spm"][:, 0:sNB], -1.0)

            # group 0 (Pool): classical then gates, as early as possible
            with tc.high_priority():
                for b in range(GROUPS[0][1], GROUPS[0][1] + GROUPS[0][2]):
                    pass1(b)
                softmax(GROUPS[0][1], GROUPS[0][1] + GROUPS[0][2])
                for b in range(GROUPS[0][1], GROUPS[0][1] + GROUPS[0][2]):
                    pass2(b)
                coeff_acts(grp[0])
                coeff_neg(grp[0])
                emit_gates(grp[0])

            # group 1 (DVE): chunk-interleaved so pass2/coeff work overlaps
            # the PE-bound pass1 phase of later chunks
            g1_lo = GROUPS[1][1]
            g1_hi = GROUPS[1][1] + GROUPS[1][2]
            bounds = [g1_lo, g1_lo + 8, g1_lo + 12, g1_hi]
            for c0, c1 in zip(bounds[:-1], bounds[1:]):
                for b in range(c0, c1):
                    pass1(b)
                softmax(c0, c1)
                for b in range(c0, c1):
                    pass2(b)
                coeff_acts(grp[1], range(c0, c1))
            coeff_neg(grp[1])
            emit_gates(grp[1])

            # ===================== LCU (one matmul per 512 chunk) ========
            # Push the tail late in the scheduler's model so the PE stream
            # never blocks on Pool/DVE gate completion mid-classical.  The
            # Pool group's LCU gets an earlier slot (Pool finishes first).
            for g, lqs in zip(grp, lqs_g):
                NB = g["NB"]
                tc.tile_set_cur_wait(0.7 if g["eng"] is nc.gpsimd else 1.0)
                cfl = cf2_h if g["dt"] == F16 else cf2_s
                w = 128 * NB
                c0 = 0
                while c0 < w:
                    cw = min(512, w - c0)
                    lp = ps_s.tile([128, 512], F32, tag="sc")
                    nc.tensor.matmul(lp[0:2, 0:cw], cfl, g["ST"][:, c0 : c0 + cw],
                                     start=True, stop=True)
                    nc.vector.tensor_copy(lqs[:, c0 : c0 + cw], lp[0:2, 0:cw])
                    c0 += cw
            tc.tile_set_cur_wait(1.0)

            # scatter rows into vTr / vTi (ri-swapped)
            for g, lqs in zip(grp, lqs_g):
                NB, b0 = g["NB"], g["b0"]
                nc.sync.dma_start(
                    out=vTr[:, b0 : b0 + NB],
                    in_=rawv(lqs[0:1, 0:1], 0, [[NB, 128], [1, NB]]))
                nc.sync.dma_start(
                    out=vTi[0:64, b0 : b0 + NB],
                    in_=rawv(lqs[1:2, 0:1], 64 * NB, [[NB, 64], [1, NB]]))
                nc.sync.dma_start(
                    out=vTi[64:128, b0 : b0 + NB],
                    in_=rawv(lqs[1:2, 0:1], 0, [[NB, 64], [1, NB]]))
            nc.vector.tensor_tensor(vT[0:64, :], vTr[0:64, :], vTi[0:64, :],
                                    ALU.subtract)
            nc.vector.tensor_tensor(vT[64:128, :], vTr[64:128, :], vTi[64:128, :],
                                    ALU.add)
            nc.vector.tensor_copy(vT_h, vT)

            # ============== 19 quadratic forms  qfeat_i = v^T M_i v ======
            t19 = ps_m.tile([128, 19 * BPC], F32, tag="m")
            for i in range(19):
                nc.tensor.matmul(t19[:, i * BPC : (i + 1) * BPC],
                                 mt_s[:, i * 128 : (i + 1) * 128], vT_h,
                                 start=True, stop=True)
            nc.vector.tensor_tensor(
                rawv(prod, 0, [[BPC, 19], [1, BPC]]),
                rawv(t19, 0, [[BPC, 19], [1, BPC]]),
                rawv(vT, 0, [[0, 19], [1, BPC]]), ALU.mult)
            qp_ps = ps_s.tile([128, 512], F32, tag="sc")
            nc.tensor.matmul(qp_ps[0:1, 0 : 19 * BPC], ones_col, prod,
                             start=True, stop=True)
            nc.vector.tensor_copy(qrow, qp_ps[0:1, 0 : 19 * BPC])
            nc.sync.dma_start(out=qfT,
                              in_=qrow.rearrange("p (a b) -> p a b", a=19, b=BPC))

            # out head: o1 = qfT^T @ owb  (row18 = ss pairs with out_b row).
            # The /ss normalization is skipped: LayerNorm is scale-invariant
            # per row, so LN(o1/ss) == LN(o1)  (ss > 0).
            o1_ps = ps_t.tile([BPC, D], F32, tag="t")
            nc.tensor.matmul(o1_ps, qfT, owb_s, start=True, stop=True)
            o1 = sm.tile([BPC, D], F32, tag="o1")
            nc.vector.tensor_copy(o1, o1_ps)

            # LayerNorm
            stats = sm.tile([BPC, 6], F32, tag="stats")
            nc.vector.bn_stats(stats, o1)
            mv = sm.tile([BPC, 2], F32, tag="mv")
            nc.vector.bn_aggr(mv, stats)
            sdv = sm.tile([BPC, 1], F32, tag="sdv")
            nc.scalar.activation(sdv, mv[:, 1:2], AF.Sqrt, bias=1e-5)
            rstd = sm.tile([BPC, 1], F32, tag="rstd")
            nc.vector.reciprocal(rstd, sdv)
            ln1 = sm.tile([BPC, D], F32, tag="ln1")
            nc.vector.tensor_scalar(ln1, o1, mv[:, 0:1], rstd,
                                    ALU.subtract, ALU.mult)

            # classifier (ln_g/ln_b folded into cw1/cb1 on host)
            lnT = [None, None]
            for h in range(2):
                lnT_ps = ps_m.tile([128, BPC], F32, tag="m")
                nc.tensor.transpose(lnT_ps, ln1[:, h * 128 : (h + 1) * 128],
                                    idn_s[0:BPC, 0:BPC])
                lnT[h] = sm.tile([128, BPC], F32, tag=f"lnT{h}", name=f"lnT{h}")
                nc.vector.tensor_copy(lnT[h], lnT_ps)
            h2p = ps_t.tile([BPC, D], F32, tag="t")
            nc.tensor.matmul(h2p, lnT[0], cw1_s[:, 0:D], start=True, stop=False)
            nc.tensor.matmul(h2p, lnT[1], cw1_s[:, D : 2 * D], start=False, stop=False)
            nc.tensor.matmul(h2p, ones[:, 0:BPC], cb1_s, start=False, stop=True)
            h2 = sm.tile([BPC, D], F32, tag="h2")
            nc.scalar.activation(h2, h2p, AF.Relu)

            h2T = [None, None]
            for h in range(2):
                h2T_ps = ps_m.tile([128, BPC], F32, tag="m")
                nc.tensor.transpose(h2T_ps, h2[:, h * 128 : (h + 1) * 128],
                                    idn_s[0:BPC, 0:BPC])
                h2T[h] = sm.tile([128, BPC], F32, tag=f"h2T{h}", name=f"h2T{h}")
                nc.vector.tensor_copy(h2T[h], h2T_ps)
            lg = ps_t.tile([BPC, 2], F32, tag="t")
            nc.tensor.matmul(lg, h2T[0], cw2_s[:, 0:2], start=True, stop=False)
            nc.tensor.matmul(lg, h2T[1], cw2_s[:, 2:4], start=False, stop=False)
            nc.tensor.matmul(lg, ones[:, 0:BPC], cb2_s, start=False, stop=True)
            lgs = sm.tile([BPC, 2], F32, tag="lgs")
            nc.vector.tensor_copy(lgs, lg)
            nc.sync.dma_start(out=out[:, :], in_=lgs)

    if split_waits:
        _split_multi_waits(nc)
    return nc


_NC_CACHE = {}


def _get_program():
    if "nc" not in _NC_CACHE:
        _NC_CACHE["nc"] = build_program()
    return _NC_CACHE["nc"]


# ----------------------------------------------------------------- host side
def _host_qff_matrices(qff_params, out_w):
    """19 symmetric 128x128 real matrices M_i = Ureal^T P_i Ureal."""
    qp = np.asarray(qff_params, np.float64)
    U = np.eye(DIM, dtype=np.complex128)

    def gate_1q(g2, wire):
        return np.kron(np.kron(np.eye(1 << wire), g2),
                       np.eye(1 << (NQ - 1 - wire)))

    def rx(t):
        c, s = np.cos(t / 2), np.sin(t / 2)
        return np.array([[c, -1j * s], [-1j * s, c]])

    def ry(t):
        c, s = np.cos(t / 2), np.sin(t / 2)
        return np.array([[c, -s], [s, c]])

    def rz(t):
        e = np.exp(-0.5j * t)
        return np.array([[e, 0], [0, np.conj(e)]])

    def crx_full(t, ctrl, tgt):
        G = np.eye(DIM, dtype=np.complex128)
        cb, tb = 5 - ctrl, 5 - tgt
        c, s = np.cos(t / 2), np.sin(t / 2)
        for a in range(DIM):
            if (a >> cb) & 1:
                G[a, a] = c
                G[a, a ^ (1 << tb)] = -1j * s
        return G

    for (kind, loc, j) in ansatz_gates(1):
        th = qp[j]
        if kind == "crx":
            G = crx_full(th, loc[0], loc[1])
        else:
            g2 = {"rx": rx, "ry": ry, "rz": rz}[kind](th)
            G = gate_1q(g2, loc)
        U = G @ U

    PX = np.array([[0, 1], [1, 0]], np.complex128)
    PY = np.array([[0, -1j], [1j, 0]], np.complex128)
    PZ = np.array([[1, 0], [0, -1]], np.complex128)

    mats = []
    for P in (PX, PY, PZ):
        for i in range(NQ):
            Pi = np.kron(np.kron(np.eye(1 << i), P), np.eye(1 << (NQ - 1 - i)))
            M = U.conj().T @ Pi @ U
            A, B = M.real, M.imag
            mats.append(np.block([[A, -B], [B, A]]))
    mats.append(np.eye(2 * DIM))
    MT = np.stack(mats, 0)  # [19, 128, 128]
    return np.ascontiguousarray(
        MT.transpose(1, 0, 2).reshape(128, 19 * 128)).astype(np.float16)


def host_prep(inputs):
    f32 = np.float32
    x = np.asarray(inputs["x"], f32)
    emb_w = np.asarray(inputs["emb_w"], np.float64)
    emb_b = np.asarray(inputs["emb_b"], np.float64)
    att_w1 = np.asarray(inputs["att_w1"], np.float64)
    att_b1 = np.asarray(inputs["att_b1"], np.float64)

    f16 = np.float16
    wfold = (emb_w @ att_w1).astype(f16)
    bfold = (emb_b @ att_w1 + att_b1).astype(f16)
    wfb = np.concatenate([wfold, bfold[None, :]], 0)

    pw = np.asarray(inputs["proj_w"], np.float64)
    pjb = np.asarray(inputs["proj_b"], np.float64)
    epw_m = np.concatenate([emb_w @ pw,
                            (emb_b @ pw + pjb)[None, :]], 0).astype(f16)

    cr = np.asarray(inputs["mix_re"], np.float64)
    ci = np.asarray(inputs["mix_im"], np.float64)
    den = np.sqrt(cr * cr + ci * ci).sum() + 1e-8
    cf2 = np.stack([cr / den, ci / den], 1).astype(f32)

    mt_m = _host_qff_matrices(inputs["qff_params"], inputs["out_w"])

    owb = np.concatenate(
        [np.asarray(inputs["out_w"], f32), np.asarray(inputs["out_b"], f32)[None, :]], 0)
    ln_g = np.asarray(inputs["ln_g"], np.float64)
    ln_b = np.asarray(inputs["ln_b"], np.float64)
    w1 = np.asarray(inputs["cls_w1"], np.float64) * ln_g[:, None]
    cw1 = np.concatenate([w1[0:128, :], w1[128:256, :]], 1).astype(f32)
    cb1 = (ln_b @ np.asarray(inputs["cls_w1"], np.float64)
           + np.asarray(inputs["cls_b1"], np.float64))[None, :].astype(f32)
    w2 = np.asarray(inputs["cls_w2"], f32)
    cw2 = np.concatenate([w2[0:128, :], w2[128:256, :]], 1)
    cb2 = np.asarray(inputs["cls_b2"], f32)[None, :]
    idn = np.eye(128, dtype=f32)

    shared = dict(wfb=wfb, aw2=np.asarray(inputs["att_w2"], f16), epw=epw_m,
                  cf2=cf2, mt=mt_m, owb=owb,
                  cw1=cw1, cb1=cb1, cw2=cw2, cb2=cb2, idn=idn)

    in_maps = []
    for c in range(N_CORES):
        xc = x[c * BPC : (c + 1) * BPC]
        # xq[b, nc, cc*16+k] = x[b, cc, nc*16+k]  (c-major)
        xq_c = np.ascontiguousarray(
            xc.reshape(BPC, C_IN, NC, CH).transpose(0, 2, 1, 3).reshape(
                BPC, NC, C_IN * CH)).astype(f16)
        m = dict(shared)
        m["xs"] = np.ascontiguousarray(xc).astype(f16)
        m["xq"] = xq_c
        in_maps.append(m)
    return in_maps


def kernel(**inputs):
    nc = _get_program()
    in_maps = host_prep(inputs)
    res = run_bass_kernel_spmd(nc, in_maps, core_ids=list(range(N_CORES)))
    outs = [res.results[c]["out"] for c in range(N_CORES)]
    return np.concatenate(outs, 0).astype(np.float32)


if __name__ == "__main__":
    nc = build_program()
    print("program built ok")


# revision 68
# speedup vs baseline: 1.0005x; 1.0005x over previous
"""Trainium2 Bass kernel for nn_ClassicalQuantumAttention (optimized).

Data-parallel over batch: 128 batch elems -> 16 per NeuronCore x 8 cores.

Quantum stage: batched state tiles ST [128 nc, 128*NB], f = q*NB + b,
q = ri*64 + a.  Each rotation gate = 4 tensor_tensor ops (2 half sigma-mults
reading permuted state views with +/-sin coefficient broadcasts, 1 full
cos-mult, 1 full add); CRX = 4 ops on the control=1 half.  Coefficients
come from co [128, 60*NB] (cos) and spm [128, 120*NB] (+sin | -sin) tiles
via stride-0 broadcast views; no per-gate expansion copies.

Pool (gpsimd) group runs batch elems 0-1 and starts right after their
classical params are ready; DVE (vector) group runs elems 2-15 in fp16.

qff ansatz + expvals folded on host into 19 symmetric 128x128 matrices;
LCU mixing is one K=128 matmul per 512-wide chunk.
"""

import numpy as np
import sys

for _p in ("/opt/trn_rl_repo",):
    if _p not in sys.path:
        sys.path.insert(0, _p)

import concourse.bass as bass
import concourse.tile as tile
from concourse import mybir
from concourse.bass_utils import run_bass_kernel_spmd

F32 = mybir.dt.float32
F16 = mybir.dt.float16
ALU = mybir.AluOpType
AF = mybir.ActivationFunctionType
AX = mybir.AxisListType

N_CORES = 8
B_TOT = 128
BPC = B_TOT // N_CORES  # 16
C_IN = 64
T = 2048
D = 256
CH = 16
NC = T // CH  # 128
NQ = 6
DIM = 64

# (engine_attr, b_start, NB, state_dtype) — Pool group first so its quantum
# stage starts as soon as its classical params are ready.  DVE NB must be
# EVEN: fp16 2x mode needs 4B-aligned unit-stride runs (runs are NB elems).
GROUPS = [("gpsimd", 0, 2, F32), ("vector", 2, 14, F16)]


def ansatz_gates(n_layers):
    gates = []
    idx = 0
    for _ in range(n_layers):
        for i in range(NQ):
            gates.append(("rx", i, idx))
            gates.append(("ry", i, idx + 1))
            gates.append(("rz", i, idx + 2))
            idx += 3
        for i in range(NQ):
            gates.append(("crx", (i, (i + 1) % NQ), idx))
            idx += 1
        for i in range(NQ - 1, -1, -1):
            gates.append(("crx", (i, (i - 1) % NQ), idx))
            idx += 1
    return gates


# --------------------------------------------------------------- AP helpers
def rawv(t, elem_off, dims):
    return bass.AP(tensor=t.tensor, offset=t.offset + elem_off,
                   ap=[list(t.ap[0])] + dims)


def mk(axes):
    """Merge (stride, count) axes outer->inner: drop count-1 axes, merge
    adjacent when outer.stride == inner.stride*inner.count."""
    dims = []
    for s, c in axes:
        if c == 1:
            continue
        if dims and dims[-1][0] == s * c:
            dims[-1] = [s, dims[-1][1] * c]
        else:
            dims.append([s, c])
    if not dims:
        dims = [[1, 1]]
    assert len(dims) <= 3, f"mk dims {dims}"
    return dims


def _split_multi_waits(nc):
    """Walrus build allows at most ONE sync-wait per instruction; hoist
    extra waits onto same-engine NoOps."""
    ctr = [0]
    for f in nc.m.functions:
        for b in f.blocks:
            new = []
            for inst in b.instructions:
                si = inst.sync_info
                if si is not None and len(si.on_wait) > 1:
                    waits = list(si.on_wait)
                    for w in waits[:-1]:
                        ctr[0] += 1
                        nop = mybir.InstNoOp(
                            name=f"wsplit-{ctr[0]}",
                            ins=[], outs=[],
                            engine=inst.engine,
                            sync_info=mybir.SyncInfo(on_wait=[w], on_update=[]),
                        )
                        new.append(nop)
                    inst.sync_info = mybir.SyncInfo(
                        on_wait=[waits[-1]], on_update=list(si.on_update)
                    )
                new.append(inst)
            b.instructions = new


# ----------------------------------------------------------- gate emission
def emit_gates(g):
    """Emit the full 2-layer ansatz for one group."""
    eng, NB = g["eng"], g["NB"]
    ST, TMP, co, spm = g["ST"], g["TMP"], g["co"], g["spm"]
    sNB = 60 * NB  # -sin block offset inside spm
    eng.memset(ST, 0.0)
    eng.memset(ST[:, 0:NB], 1.0)

    def sview(t, p, L, ri=None, k=None, ri_neg=False, k_neg=False):
        off = 0
        axes = []
        if ri is None:
            if ri_neg:
                axes.append((-64 * NB, 2))
                off += 64 * NB
            else:
                axes.append((64 * NB, 2))
        else:
            off += ri * 64 * NB
        axes.append(((1 << (p + 1)) * NB, 1 << (5 - p)))
        if k is None:
            s = (1 << p) * NB
            if k_neg:
                axes.append((-s, 2))
                off += s
            else:
                axes.append((s, 2))
        else:
            off += k * (1 << p) * NB
        axes.append((1, L))
        return rawv(t, off, mk(axes))

    def cf(tile_, base, rest, slot=None):
        dims = []
        if slot is not None:
            dims.append(list(slot))
        if rest > 1:
            dims.append([0, rest])
        dims.append([1, NB])
        return rawv(tile_, base, dims)

    def crx_view(t, pc, pt, ri, kt_neg):
        off = (1 << pc) * NB
        axes = []
        if ri is None:
            axes.append((64 * NB, 2))
        else:
            off += ri * 64 * NB
        for bit in range(5, -1, -1):
            if bit == pc:
                continue
            s = (1 << bit) * NB
            if bit == pt and kt_neg:
                axes.append((-s, 2))
                off += s
            else:
                axes.append((s, 2))
        axes.append((1, NB))
        return rawv(t, off, mk(axes))

    for gi, (kind, loc, j) in enumerate(ansatz_gates(2)):
        jb = j * NB
        if kind != "crx":
            p = 5 - loc
            ah = (gi // 3) + 1 if gi < 18 else 6
            la = p - (6 - ah)
            assert la >= 0
            L = NB << la
            H = 1 << (5 - p)
            LN_ = L // NB  # lo amp count
            expand = (kind == "rx" and gi < 18)
            if kind == "rx":
                for ri_o in (0, 1):
                    ov = sview(TMP, p, L, ri=ri_o, k=(1 if expand else None))
                    iv = sview(ST, p, L, ri=1 - ri_o,
                               k=(0 if expand else None),
                               k_neg=not expand)
                    c = cf(spm, jb + (0 if ri_o == 0 else sNB),
                           H * (1 if expand else 2) * LN_)
                    eng.tensor_tensor(ov, iv, c, ALU.mult)
            elif kind == "ry":
                for k_o in (0, 1):
                    ov = sview(TMP, p, L, k=k_o)
                    iv = sview(ST, p, L, k=1 - k_o)
                    c = cf(spm, jb + (sNB if k_o == 0 else 0), 2 * H * LN_)
                    eng.tensor_tensor(ov, iv, c, ALU.mult)
            else:  # rz
                for k_o in (0, 1):
                    ov = sview(TMP, p, L, k=k_o)
                    iv = sview(ST, p, L, k=k_o, ri_neg=True)
                    base = jb + (0 if k_o == 0 else sNB)
                    slot = [sNB if k_o == 0 else -sNB, 2]
                    c = cf(spm, base, H * LN_, slot=slot)
                    eng.tensor_tensor(ov, iv, c, ALU.mult)
            if expand:
                cm = sview(ST, p, L, k=0)
                eng.tensor_tensor(cm, cm, cf(co, jb, 2 * H * LN_), ALU.mult)
                ao = sview(ST, p, L, k=1)
                ai = sview(TMP, p, L, k=1)
                eng.tensor_tensor(ao, ao, ai, ALU.add)
            else:
                cm = sview(ST, p, L)
                eng.tensor_tensor(cm, cm, cf(co, jb, 4 * H * LN_), ALU.mult)
                ao = sview(ST, p, L)
                ai = sview(TMP, p, L)
                eng.tensor_tensor(ao, ao, ai, ALU.add)
        else:
            wc, wt = loc
            pc, pt = 5 - wc, 5 - wt
            for ri_o in (0, 1):
                ov = crx_view(TMP, pc, pt, ri_o, False)
                iv = crx_view(ST, pc, pt, 1 - ri_o, True)
                c = cf(spm, jb + (0 if ri_o == 0 else sNB), 32)
                eng.tensor_tensor(ov, iv, c, ALU.mult)
            cm = crx_view(ST, pc, pt, None, False)
            eng.tensor_tensor(cm, cm, cf(co, jb, 64), ALU.mult)
            ao = crx_view(ST, pc, pt, None, False)
            ai = crx_view(TMP, pc, pt, None, False)
            eng.tensor_tensor(ao, ao, ai, ALU.add)


# ----------------------------------------------------------------- program
def build_program(split_waits=True):
    nc = bass.Bass()

    for vconst in (float(np.pi / 2), 1e-5, 0.25, float(0.25 + np.pi / 2)):
        t = nc.alloc_sbuf_tensor(f"const-f32-{vconst}", [128, 1], F32)
        nc.gpsimd.memset(t.ap(), vconst)
        nc.const_aps.aps[(F32, vconst)] = t.ap()
    nc.all_engine_barrier()

    xs = nc.declare_dram_parameter("xs", [BPC, C_IN, T], F16, isOutput=False)
    xq = nc.declare_dram_parameter("xq", [BPC, NC, CH * C_IN], F16, isOutput=False)
    wfb = nc.declare_dram_parameter("wfb", [C_IN + 1, 128], F16, isOutput=False)
    aw2 = nc.declare_dram_parameter("aw2", [128, 1], F16, isOutput=False)
    epw = nc.declare_dram_parameter("epw", [C_IN + 1, 60], F16, isOutput=False)
    cf2 = nc.declare_dram_parameter("cf2", [NC, 2], F32, isOutput=False)
    mt = nc.declare_dram_parameter("mt", [128, 19 * 128], F16, isOutput=False)
    owb = nc.declare_dram_parameter("owb", [19, D], F32, isOutput=False)
    cw1 = nc.declare_dram_parameter("cw1", [128, 2 * D], F32, isOutput=False)
    cb1 = nc.declare_dram_parameter("cb1", [1, D], F32, isOutput=False)
    cw2 = nc.declare_dram_parameter("cw2", [128, 4], F32, isOutput=False)
    cb2 = nc.declare_dram_parameter("cb2", [1, 2], F32, isOutput=False)
    idn = nc.declare_dram_parameter("idn", [128, 128], F32, isOutput=False)
    out = nc.declare_dram_parameter("out", [BPC, 2], F32, isOutput=True)

    with tile.TileContext(nc) as tc:
        with (
            tc.tile_pool(name="const", bufs=1) as cp,
            tc.tile_pool(name="xbuf", bufs=4) as xpool,
            tc.tile_pool(name="xqbuf", bufs=4) as xqpool,
            tc.tile_pool(name="tanh", bufs=6) as thpool,
            tc.tile_pool(name="sscb", bufs=3) as sscpool,
            tc.tile_pool(name="small", bufs=8) as sm,
            tc.tile_pool(name="ps_h", bufs=2, space="PSUM") as ps_h,
            tc.tile_pool(name="ps_s", bufs=2, space="PSUM") as ps_s,
            tc.tile_pool(name="ps_m", bufs=2, space="PSUM") as ps_m,
            tc.tile_pool(name="ps_t", bufs=1, space="PSUM") as ps_t,
            tc.tile_pool(name="ps_x", bufs=1, space="PSUM") as ps_x,
        ):
            def cload(name, dram, shape, dt=F32):
                t = cp.tile(shape, dt, tag=name, name=name)
                nc.sync.dma_start(out=t, in_=dram[:, :])
                return t

            wfb_s = cload("wfb", wfb, [C_IN + 1, 128], F16)
            aw2_s = cload("aw2", aw2, [128, 1], F16)
            epw_s = cload("epw", epw, [C_IN + 1, 60], F16)
            idn_s = cload("idn", idn, [128, 128])
            idn_h = cp.tile([128, 128], F16, tag="idn_h")
            nc.vector.tensor_copy(idn_h, idn_s)
            # tail-only weights: keep their DMAs off the startup window
            with tc.tile_wait_until(0.5):
                cf2_s = cload("cf2", cf2, [NC, 2])
                mt_s = cload("mt", mt, [128, 19 * 128], F16)
                owb_s = cload("owb", owb, [19, D])
                cw1_s = cload("cw1", cw1, [128, 2 * D])
                cb1_s = cload("cb1", cb1, [1, D])
                cw2_s = cload("cw2", cw2, [128, 4])
                cb2_s = cload("cb2", cb2, [1, 2])

            ones = cp.tile([1, 128], F32, tag="ones")
            nc.vector.memset(ones, 1.0)
            ones_col = cp.tile([128, 1], F32, tag="ones_col")
            nc.vector.memset(ones_col, 1.0)

            sc_all = cp.tile([NC, BPC * CH], F16, tag="sc_all", name="sc_all")
            esc = cp.tile([NC, BPC * CH], F32, tag="esc", name="esc")
            w_all = cp.tile([NC, BPC * CH], F16, tag="w_all", name="w_all")

            # per-engine-group quantum tiles
            grp = []
            for gi_, (engname, b0, NB, sdt) in enumerate(GROUPS):
                g = dict(
                    eng=getattr(nc, engname), b0=b0, NB=NB, dt=sdt,
                    ST=cp.tile([128, 128 * NB], sdt, tag=f"ST{gi_}", name=f"ST{gi_}"),
                    TMP=cp.tile([128, 128 * NB], sdt, tag=f"TMP{gi_}", name=f"TMP{gi_}"),
                    co=cp.tile([128, 60 * NB], sdt, tag=f"co{gi_}", name=f"co{gi_}"),
                    spm=cp.tile([128, 120 * NB], sdt, tag=f"spm{gi_}", name=f"spm{gi_}"),
                )
                grp.append(g)
            cf2_h = cp.tile([NC, 2], F16, tag="cf2h")
            with tc.tile_wait_until(0.5):
                nc.vector.tensor_copy(cf2_h, cf2_s)

            x_sb = [xpool.tile([C_IN + 1, T], F16, tag="x", name=f"xsb{i}") for i in range(4)]
            xq_sb = [xqpool.tile([NC, CH * C_IN], F16, tag="xq", name=f"xqsb{i}") for i in range(4)]
            nc.vector.memset(x_sb[0][C_IN : C_IN + 1, :], 1.0)
            for i in (1, 2, 3):
                nc.gpsimd.memset(x_sb[i][C_IN : C_IN + 1, :], 1.0)

            xwt_sb = [sm.tile([C_IN + 1, NC], F16, tag=f"xwt{i}", name=f"xwt{i}") for i in range(3)]
            for i in range(3):
                nc.vector.memset(xwt_sb[i][C_IN : C_IN + 1, :], 1.0)

            lqs_g = [cp.tile([2, 128 * nbv], F32, tag=f"lqs{i_}", name=f"lqs{i_}")
                     for i_, (_, _, nbv, _) in enumerate(GROUPS)]
            vTr = cp.tile([128, BPC], F32, tag="vTr")
            vTi = cp.tile([128, BPC], F32, tag="vTi")
            vT = cp.tile([128, BPC], F32, tag="vT")
            vT_h = cp.tile([128, BPC], F16, tag="vTh")
            prod = cp.tile([128, 19 * BPC], F32, tag="prod")
            qrow = cp.tile([1, 19 * BPC], F32, tag="qrow")
            qfT = cp.tile([19, BPC], F32, tag="qfT")
            ssc_n = cp.tile([BPC, 1], F32, tag="ssc")
            rss = cp.tile([BPC, 1], F32, tag="rss")

            par_sb = [None] * BPC

            # ===================== classical passes ======================
            def pass1(b):
                xb = x_sb[b % 4]
                nc.sync.dma_start(out=xb[0:C_IN, :], in_=xs[b, :, :])
                # score rows land on PSUM partitions 0/32 of two tiles; each
                # pair is moved by ONE cast (partitions run in parallel on DVE)
                sc_ps = [None, None]
                ssc_t = [None, None]
                for half in range(2):
                    sc_ps[half] = ps_s.tile([128, 512], F32, tag="sc",
                                            name=f"scps{half}")
                    ssc_t[half] = sscpool.tile([33, 512], F16, tag=f"ss{half}",
                                               name=f"ss{half}")
                for blk in range(4):
                    hp = ps_h.tile([128, 512], F32, tag="hp")
                    nc.tensor.matmul(hp, wfb_s, xb[:, blk * 512 : (blk + 1) * 512],
                                     start=True, stop=True)
                    th = thpool.tile([128, 512], F16, tag="th")
                    nc.scalar.activation(th, hp, AF.Tanh)
                    nc.tensor.matmul(sc_ps[blk // 2][32 * (blk % 2) : 32 * (blk % 2) + 1, :],
                                     aw2_s, th, start=True, stop=True)
                for half in range(2):
                    nc.vector.tensor_copy(ssc_t[half], sc_ps[half][0:33, :])
                for half in range(2):
                    # one DMA covers both score rows (partition-strided src)
                    st = ssc_t[half]
                    nc.sync.dma_start(
                        out=sc_all[64 * half : 64 * (half + 1),
                                   b * CH : (b + 1) * CH],
                        in_=bass.AP(tensor=st.tensor, offset=st.offset,
                                    ap=[[512 * 32, 2], [CH, 32], [1, CH]]))

            def softmax(b_lo, b_hi):
                nb = b_hi - b_lo
                lo, hi = b_lo * CH, b_hi * CH
                nc.scalar.activation(esc[:, lo:hi], sc_all[:, lo:hi], AF.Exp)
                ssum = sm.tile([NC, BPC], F32, tag="ssum")
                nc.vector.tensor_reduce(
                    ssum[:, 0:nb],
                    esc[:, lo:hi].rearrange("p (n k) -> p n k", n=nb, k=CH),
                    AX.X, ALU.add)
                rsum = sm.tile([NC, BPC], F32, tag="rsum")
                nc.vector.reciprocal(rsum[:, 0:nb], ssum[:, 0:nb])
                for bb in range(nb):
                    b = b_lo + bb
                    nc.vector.tensor_scalar_mul(
                        w_all[:, b * CH : (b + 1) * CH],
                        esc[:, b * CH : (b + 1) * CH],
                        rsum[:, bb : bb + 1])

            def pass2(b):
                xqb = xq_sb[b % 4]
                nc.sync.dma_start(out=xqb, in_=xq[b, :, :])
                # early elems ride the Pool engine (idle until its gates);
                # later ones use DVE
                xw_eng = nc.vector
                tmpxw = sm.tile([NC, CH * C_IN], F16, tag="tmpxw")
                wv = rawv(w_all, b * CH, [[0, C_IN], [1, CH]])
                xw_eng.tensor_tensor(
                    rawv(tmpxw, 0, [[CH, C_IN], [1, CH]]),
                    rawv(xqb, 0, [[CH, C_IN], [1, CH]]),
                    wv, ALU.mult)
                xw = sm.tile([NC, C_IN], F16, tag="xw")
                with nc.allow_low_precision(reason="16-wide sum, tol 2e-2"):
                    xw_eng.tensor_reduce(
                        xw, tmpxw.rearrange("p (c k) -> p c k", c=C_IN, k=CH),
                        AX.X, ALU.add)
                xwt_ps = ps_x.tile([C_IN, NC], F16, tag="mh")
                nc.tensor.transpose(xwt_ps, xw, idn_h)
                xwt = xwt_sb[b % 3]
                nc.scalar.copy(xwt[0:C_IN, :], xwt_ps)
                par = ps_t.tile([NC, 60], F32, tag="t")
                nc.tensor.matmul(par, xwt, epw_s, start=True, stop=True)
                par_s = sm.tile([NC, 60], F32, tag=f"pars{b % 8}", name=f"pars{b % 8}")
                # par_s holds tanh(p/2); sigmoid(p) = (1+tanh(p/2))/2 is
                # folded into the Sin activations (Tanh table stays resident,
                # avoiding Sigmoid table swaps mid-stream).
                nc.scalar.activation(par_s, par, AF.Tanh, scale=0.5)
                par_sb[b] = par_s

            def coeff_acts(g, bs=None):
                """sin/cos coefficient tiles for group batch elems (by func)."""
                NB, b0 = g["NB"], g["b0"]
                if bs is None:
                    bs = range(b0, b0 + NB)
                for b in bs:
                    nc.scalar.activation(
                        rawv(g["co"], b - b0, [[NB, 60]]), par_sb[b],
                        AF.Sin, bias=float(0.25 + np.pi / 2), scale=0.25)
                for b in bs:
                    nc.scalar.activation(
                        rawv(g["spm"], b - b0, [[NB, 60]]), par_sb[b],
                        AF.Sin, bias=0.25, scale=0.25)

            def coeff_neg(g):
                # -sin half of spm in one engine-side negate (off scalar)
                NB = g["NB"]
                sNB = 60 * NB
                g["eng"].tensor_scalar_mul(
                    g["spm"][:, sNB : 2 * sNB], g["spm"][:, 0:sNB], -1.0)

            # group 0 (Pool): classical then gates, as early as possible
            with tc.high_priority():
                for b in range(GROUPS[0][1], GROUPS[0][1] + GROUPS[0][2]):
                    pass1(b)
                softmax(GROUPS[0][1], GROUPS[0][1] + GROUPS[0][2])
                for b in range(GROUPS[0][1], GROUPS[0][1] + GROUPS[0][2]):
                    pass2(b)
                coeff_acts(grp[0])
                coeff_neg(grp[0])
                emit_gates(grp[0])

            # group 1 (DVE): chunk-interleaved so pass2/coeff work overlaps
            # the PE-bound pass1 phase of later chunks
            g1_lo = GROUPS[1][1]
            g1_hi = GROUPS[1][1] + GROUPS[1][2]
            bounds = [g1_lo, g1_lo + 8, g1_lo + 12, g1_hi]
            for c0, c1 in zip(bounds[:-1], bounds[1:]):
                for b in range(c0, c1):
                    pass1(b)
                softmax(c0, c1)
                for b in range(c0, c1):
                    pass2(b)
                coeff_acts(grp[1], range(c0, c1))
            coeff_neg(grp[1])
            emit_gates(grp[1])

            # ===================== LCU (one matmul per 512 chunk) ========
            # Push the tail late in the scheduler's model so the PE stream
            # never blocks on Pool/DVE gate completion mid-classical.  The
            # Pool group's LCU gets an earlier slot (Pool finishes first).
            for g, lqs in zip(grp, lqs_g):
                NB = g["NB"]
                tc.tile_set_cur_wait(0.7 if g["eng"] is nc.gpsimd else 1.0)
                cfl = cf2_h if g["dt"] == F16 else cf2_s
                w = 128 * NB
                c0 = 0
                while c0 < w:
                    cw = min(512, w - c0)
                    lp = ps_s.tile([128, 512], F32, tag="sc")
                    nc.tensor.matmul(lp[0:2, 0:cw], cfl, g["ST"][:, c0 : c0 + cw],
                                     start=True, stop=True)
                    nc.vector.tensor_copy(lqs[:, c0 : c0 + cw], lp[0:2, 0:cw])
                    c0 += cw
            tc.tile_set_cur_wait(1.0)

            # scatter rows into vTr / vTi (ri-swapped)
            for g, lqs in zip(grp, lqs_g):
                NB, b0 = g["NB"], g["b0"]
                nc.sync.dma_start(
                    out=vTr[:, b0 : b0 + NB],
                    in_=rawv(lqs[0:1, 0:1], 0, [[NB, 128], [1, NB]]))
                nc.sync.dma_start(
                    out=vTi[0:64, b0 : b0 + NB],
                    in_=rawv(lqs[1:2, 0:1], 64 * NB, [[NB, 64], [1, NB]]))
                nc.sync.dma_start(
                    out=vTi[64:128, b0 : b0 + NB],
                    in_=rawv(lqs[1:2, 0:1], 0, [[NB, 64], [1, NB]]))
            nc.vector.tensor_tensor(vT[0:64, :], vTr[0:64, :], vTi[0:64, :],
                                    ALU.subtract)
            nc.vector.tensor_tensor(vT[64:128, :], vTr[64:128, :], vTi[64:128, :],
                                    ALU.add)
            nc.vector.tensor_copy(vT_h, vT)

            # ============== 19 quadratic forms  qfeat_i = v^T M_i v ======
            t19 = ps_m.tile([128, 19 * BPC], F32, tag="m")
            for i in range(19):
                nc.tensor.matmul(t19[:, i * BPC : (i + 1) * BPC],
                                 mt_s[:, i * 128 : (i + 1) * 128], vT_h,
                                 start=True, stop=True)
            nc.vector.tensor_tensor(
                rawv(prod, 0, [[BPC, 19], [1, BPC]]),
                rawv(t19, 0, [[BPC, 19], [1, BPC]]),
                rawv(vT, 0, [[0, 19], [1, BPC]]), ALU.mult)
            qp_ps = ps_s.tile([128, 512], F32, tag="sc")
            nc.tensor.matmul(qp_ps[0:1, 0 : 19 * BPC], ones_col, prod,
                             start=True, stop=True)
            nc.vector.tensor_copy(qrow, qp_ps[0:1, 0 : 19 * BPC])
            nc.sync.dma_start(out=qfT,
                              in_=qrow.rearrange("p (a b) -> p a b", a=19, b=BPC))

            # out head: o1 = qfT^T @ owb  (row18 = ss pairs with out_b row).
            # The /ss normalization is skipped: LayerNorm is scale-invariant
            # per row, so LN(o1/ss) == LN(o1)  (ss > 0).
            o1_ps = ps_t.tile([BPC, D], F32, tag="t")
            nc.tensor.matmul(o1_ps, qfT, owb_s, start=True, stop=True)
            o1 = sm.tile([BPC, D], F32, tag="o1")
            nc.vector.tensor_copy(o1, o1_ps)

            # LayerNorm
            stats = sm.tile([BPC, 6], F32, tag="stats")
            nc.vector.bn_stats(stats, o1)
            mv = sm.tile([BPC, 2], F32, tag="mv")
            nc.vector.bn_aggr(mv, stats)
            sdv = sm.tile([BPC, 1], F32, tag="sdv")
            nc.scalar.activation(sdv, mv[:, 1:2], AF.Sqrt, bias=1e-5)
            rstd = sm.tile([BPC, 1], F32, tag="rstd")
            nc.vector.reciprocal(rstd, sdv)
            ln1 = sm.tile([BPC, D], F32, tag="ln1")
            nc.vector.tensor_scalar(ln1, o1, mv[:, 0:1], rstd,
                                    ALU.subtract, ALU.mult)

            # classifier (ln_g/ln_b folded into cw1/cb1 on host)
            lnT = [None, None]
            for h in range(2):
                lnT_ps = ps_m.tile([128, BPC], F32, tag="m")
                nc.tensor.transpose(lnT_ps, ln1[:, h * 128 : (h + 1) * 128],
                                    idn_s[0:BPC, 0:BPC])
                lnT[h] = sm.tile([128, BPC], F32, tag=f"lnT{h}", name=f"lnT{h}")
                nc.vector.tensor_copy(lnT[h], lnT_ps)
            h2p = ps_t.tile([BPC, D], F32, tag="t")
            nc.tensor.matmul(h2p, lnT[0], cw1_s[:, 0:D], start=True, stop=False)
            nc.tensor.matmul(h2p, lnT[1], cw1_s[:, D : 2 * D], start=False, stop=False)
            nc.tensor.matmul(h2p, ones[:, 0:BPC], cb1_s, start=False, stop=True)
            h2 = sm.tile([BPC, D], F32, tag="h2")
            nc.scalar.activation(h2, h2p, AF.Relu)

            h2T = [None, None]
            for h in range(2):
                h2T_ps = ps_m.tile([128, BPC], F32, tag="m")
                nc.tensor.transpose(h2T_ps, h2[:, h * 128 : (h + 1) * 128],
                                    idn_s[0:BPC, 0:BPC])
                h2T[h] = sm.tile([128, BPC], F32, tag=f"h2T{h}", name=f"h2T{h}")
                nc.vector.tensor_copy(h2T[h], h2T_ps)
            lg = ps_t.tile([BPC, 2], F32, tag="t")
            nc.tensor.matmul(lg, h2T[0], cw2_s[:, 0:2], start=True, stop=False)
            nc.tensor.matmul(lg, h2T[1], cw2_s[:, 2:4], start=False, stop=False)
            nc.tensor.matmul(lg, ones[:, 0:BPC], cb2_s, start=False, stop=True)
            lgs = sm.tile([BPC, 2], F32, tag="lgs")
            nc.vector.tensor_copy(lgs, lg)
            nc.sync.dma_start(out=out[:, :], in_=lgs)

    if split_waits:
        _split_multi_waits(nc)
    return nc


_NC_CACHE = {}


def _get_program():
    if "nc" not in _NC_CACHE:
        _NC_CACHE["nc"] = build_program()
    return _NC_CACHE["nc"]


# ----------------------------------------------------------------- host side
def _host_qff_matrices(qff_params, out_w):
    """19 symmetric 128x128 real matrices M_i = Ureal^T P_i Ureal."""
    qp = np.asarray(qff_params, np.float64)
    U = np.eye(DIM, dtype=np.complex128)

    def gate_1q(g2, wire):
        return np.kron(np.kron(np.eye(1 << wire), g2),
                       np.eye(1 << (NQ - 1 - wire)))

    def rx(t):
        c, s = np.cos(t / 2), np.sin(t / 2)
        return np.array([[c, -1j * s], [-1j * s, c]])

    def ry(t):
        c, s = np.cos(t / 2), np.sin(t / 2)
        return np.array([[c, -s], [s, c]])

    def rz(t):
        e = np.exp(-0.5j * t)
        return np.array([[e, 0], [0, np.conj(e)]])

    def crx_full(t, ctrl, tgt):
        G = np.eye(DIM, dtype=np.complex128)
        cb, tb = 5 - ctrl, 5 - tgt
        c, s = np.cos(t / 2), np.sin(t / 2)
        for a in range(DIM):
            if (a >> cb) & 1:
                G[a, a] = c
                G[a, a ^ (1 << tb)] = -1j * s
        return G

    for (kind, loc, j) in ansatz_gates(1):
        th = qp[j]
        if kind == "crx":
            G = crx_full(th, loc[0], loc[1])
        else:
            g2 = {"rx": rx, "ry": ry, "rz": rz}[kind](th)
            G = gate_1q(g2, loc)
        U = G @ U

    PX = np.array([[0, 1], [1, 0]], np.complex128)
    PY = np.array([[0, -1j], [1j, 0]], np.complex128)
    PZ = np.array([[1, 0], [0, -1]], np.complex128)

    mats = []
    for P in (PX, PY, PZ):
        for i in range(NQ):
            Pi = np.kron(np.kron(np.eye(1 << i), P), np.eye(1 << (NQ - 1 - i)))
            M = U.conj().T @ Pi @ U
            A, B = M.real, M.imag
            mats.append(np.block([[A, -B], [B, A]]))
    mats.append(np.eye(2 * DIM))
    MT = np.stack(mats, 0)  # [19, 128, 128]
    return np.ascontiguousarray(
        MT.transpose(1, 0, 2).reshape(128, 19 * 128)).astype(np.float16)


def host_prep(inputs):
    f32 = np.float32
    x = np.asarray(inputs["x"], f32)
    emb_w = np.asarray(inputs["emb_w"], np.float64)
    emb_b = np.asarray(inputs["emb_b"], np.float64)
    att_w1 = np.asarray(inputs["att_w1"], np.float64)
    att_b1 = np.asarray(inputs["att_b1"], np.float64)

    f16 = np.float16
    wfold = (emb_w @ att_w1).astype(f16)
    bfold = (emb_b @ att_w1 + att_b1).astype(f16)
    wfb = np.concatenate([wfold, bfold[None, :]], 0)

    pw = np.asarray(inputs["proj_w"], np.float64)
    pjb = np.asarray(inputs["proj_b"], np.float64)
    epw_m = np.concatenate([emb_w @ pw,
                            (emb_b @ pw + pjb)[None, :]], 0).astype(f16)

    cr = np.asarray(inputs["mix_re"], np.float64)
    ci = np.asarray(inputs["mix_im"], np.float64)
    den = np.sqrt(cr * cr + ci * ci).sum() + 1e-8
    cf2 = np.stack([cr / den, ci / den], 1).astype(f32)

    mt_m = _host_qff_matrices(inputs["qff_params"], inputs["out_w"])

    owb = np.concatenate(
        [np.asarray(inputs["out_w"], f32), np.asarray(inputs["out_b"], f32)[None, :]], 0)
    ln_g = np.asarray(inputs["ln_g"], np.float64)
    ln_b = np.asarray(inputs["ln_b"], np.float64)
    w1 = np.asarray(inputs["cls_w1"], np.float64) * ln_g[:, None]
    cw1 = np.concatenate([w1[0:128, :], w1[128:256, :]], 1).astype(f32)
    cb1 = (ln_b @ np.asarray(inputs["cls_w1"], np.float64)
           + np.asarray(inputs["cls_b1"], np.float64))[None, :].astype(f32)
    w2 = np.asarray(inputs["cls_w2"], f32)
    cw2 = np.concatenate([w2[0:128, :], w2[128:256, :]], 1)
    cb2 = np.asarray(inputs["cls_b2"], f32)[None, :]
    idn = np.eye(128, dtype=f32)

    shared = dict(wfb=wfb, aw2=np.asarray(inputs["att_w2"], f16), epw=epw_m,
                  cf2=cf2, mt=mt_m, owb=owb,
                  cw1=cw1, cb1=cb1, cw2=cw2, cb2=cb2, idn=idn)

    in_maps = []
    for c in range(N_CORES):
        xc = x[c * BPC : (c + 1) * BPC]
        # xq[b, nc, cc*16+k] = x[b, cc, nc*16+k]  (c-major)
        xq_c = np.ascontiguousarray(
            xc.reshape(BPC, C_IN, NC, CH).transpose(0, 2, 1, 3).reshape(
                BPC, NC, C_IN * CH)).astype(f16)
        m = dict(shared)
        m["xs"] = np.ascontiguousarray(xc).astype(f16)
        m["xq"] = xq_c
        in_maps.append(m)
    return in_maps


def kernel(**inputs):
    nc = _get_program()
    in_maps = host_prep(inputs)
    res = run_bass_kernel_spmd(nc, in_maps, core_ids=list(range(N_CORES)))
    outs = [res.results[c]["out"] for c in range(N_CORES)]
    return np.concatenate(outs, 0).astype(np.float32)


if __name__ == "__main__":
    nc = build_program()
    print("program built ok")
